# revision 41
# baseline (speedup 1.0000x reference)
"""Bahdanau-attention kernel for Trainium2 (8 NeuronCores, data-parallel over batch).

Computation (per batch b):
    enc_proj = h_enc @ W1.T + b1          # (L, D)   -- the big matmul
    dec_proj = h_dec @ W2.T + b2          # (D,)  -- computed on host (tiny)
    h        = tanh(enc_proj + dec_proj)  # (L, D)
    scores   = h @ V (+ bv)               # (L,)  -- bv cancels in softmax, dropped
    attn     = softmax(scores)            # no-max softmax: |scores| <= ||V||_1 ~ 16, exp is safe
    ctx      = attn @ enc_proj            # (D,)

Device layout: everything transposed ("T-space", e/d on partitions):
  - h_enc cast to fp16 AND pre-transposed on HOST into the exact SBUF
    layout [b, q, dpart, t, dchunk, l]; the device just streams one
    contiguous 1MB DMA per l-quarter (no on-device transpose)
  - dec_proj (+b1+b2) computed on host (67 MFLOP numpy) -> "biast" input;
    no W2/h_dec on device at all
  - q0/q1 tiles load as 2 half-tiles each on the otherwise-idle sync HW
    ring (low latency); steady-state tiles on the gpsimd/SWDGE ring whose
    spread-out traffic does not disturb the PE clock (HW-DGE bulk does),
    gated behind the startup-critical loads via dummy-buffer WAW deps
  - L processed in quarters (512): load -> 8x8 matmuls (PE) ->
    tanh+evac (ACT) -> scores (PE) -> exp (ACT) -> ctx partial (DVE)
  - W1 relaid per-output-chunk and loaded as per-chunk DMAs so chunk c's
    semaphore fires as soon as its own bytes land
  - tanh fused with (host dec_proj+b1+b2) bias on ACT; exp fused with
    Z-sum on ACT
  - scores via PE with V replicated to 128 rows -> replicated scores for free
  - ctx via one fused DVE scalar_tensor_tensor (product + free-axis
    accumulate) per (chunk, quarter) against the evacuated enc_projT
  - softmax division and final quarter-sums happen on the host (raw
    per-quarter ctx/Z partials are the kernel outputs)
"""

import numpy as np

B, L, D = 32, 2048, 1024
NCORES = 8
NB = B // NCORES  # batches per core
P = 128
NCH = D // P      # 8 chunks of the d/e dimension
NQ = 4            # l-quarters per batch
LQ = L // NQ      # 512
TQ = LQ // P      # 4 l-subtiles per quarter

_cache = {}


def _build(reps=1):
    import concourse.bass as bass
    import concourse.tile as tile
    from concourse import bacc, mybir
    from concourse.bass import ts, ds
    from contextlib import ExitStack

    FP16 = mybir.dt.float16
    FP32 = mybir.dt.float32
    Alu = mybir.AluOpType
    Act = mybir.ActivationFunctionType
    X = mybir.AxisListType.X

    nc = bacc.Bacc("TRN2", name="bahdanau_attn")

    # host-pretransposed encoder states: ht[b, q, dp, t, dc, l]
    ht = nc.dram_tensor("ht", [NB, NQ, P, TQ, NCH, P], FP16, kind="ExternalInput")
    # per-output-chunk weight layout: w1r[c][dpart, dchunk, e'] = W1T[d, c*128+e']
    w1r = nc.dram_tensor("w1r", [NCH, P, NCH, P], FP16, kind="ExternalInput")
    biast = nc.dram_tensor("biast", [P, NCH, NB], FP32, kind="ExternalInput")  # dec_proj+b1+b2
    b1t = nc.dram_tensor("b1t", [P, NCH], FP32, kind="ExternalInput")          # b1 as [p, chunk]
    vt = nc.dram_tensor("vt", [P, NCH], FP16, kind="ExternalInput")            # V as [p, chunk]
    # raw per-quarter ctx partials and Z partials; host does sum + divide
    out = nc.dram_tensor("ctx_out", [NB, P, NCH, NQ], FP32, kind="ExternalOutput")
    zout = nc.dram_tensor("z_out", [NB, P, NQ], FP32, kind="ExternalOutput")

    with tile.TileContext(nc) as tc, ExitStack() as ctx:
        wp = ctx.enter_context(tc.tile_pool(name="weights", bufs=1))
        # small buffer counts on the h-tile pools are load-bearing: the SWDGE
        # ring's transfers gate on buffer release, which keeps the early HBM
        # fabric dedicated to the startup-critical q0/q1/weight loads
        tpp = ctx.enter_context(tc.tile_pool(name="hTpieces", bufs=2))
        q1p = ctx.enter_context(tc.tile_pool(name="hTq1", bufs=1))
        tp = ctx.enter_context(tc.tile_pool(name="hT", bufs=4))
        ep = ctx.enter_context(tc.tile_pool(name="encproj", bufs=2))
        hp = ctx.enter_context(tc.tile_pool(name="htan", bufs=4))
        xp = ctx.enter_context(tc.tile_pool(name="exps", bufs=2))
        sp = ctx.enter_context(tc.tile_pool(name="scratch", bufs=3))
        fin = ctx.enter_context(tc.tile_pool(name="final", bufs=2))
        psA = ctx.enter_context(tc.tile_pool(name="psA", bufs=5, space="PSUM"))
        psS = ctx.enter_context(tc.tile_pool(name="psS", bufs=2, space="PSUM"))

        w1_sb = wp.tile([P, NCH, NCH, P], FP16)  # [dpart, cchunk, dchunk, e']

        # first quarter's h_encT tile in 2 pieces on the idle sync HW ring
        # q0 as 2 half-tiles + q1 as one tile on the sync HW ring (serial
        # FIFO; SWDGE's ~20us cold-start latency can't feed the PE this
        # early). q0's halves let the PE start ~3us sooner; q1 arrives
        # before the PE reaches it either way, so full-width is cheaper.
        hT0a = tpp.tile([P, TQ // 2, NCH, P], FP16, tag="hTa")
        hT0b = tpp.tile([P, TQ // 2, NCH, P], FP16, tag="hTb")
        hT1a = tpp.tile([P, TQ // 2, NCH, P], FP16, tag="hTa")
        hT1b = tpp.tile([P, TQ // 2, NCH, P], FP16, tag="hTb")
        nc.sync.dma_start(hT0a, ht[0, 0, :, ds(0, TQ // 2)])
        nc.sync.dma_start(hT0b, ht[0, 0, :, ds(TQ // 2, TQ // 2)])
        nc.sync.dma_start(hT1a, ht[0, 1, :, ds(0, TQ // 2)])
        nc.sync.dma_start(hT1b, ht[0, 1, :, ds(TQ // 2, TQ // 2)])

        # prologue loads (scalar ring), critical-path first; w1 as per-chunk
        # DMAs so chunk c's semaphore fires as soon as ITS bytes land (a
        # single bulk DMA would gate c=3..7 on the full 2MB)
        bias_sb = wp.tile([P, NCH, NB], FP32)
        nc.scalar.dma_start(bias_sb, biast[:])
        b1_sb = wp.tile([P, NCH], FP32)
        nc.scalar.dma_start(b1_sb, b1t[:])
        v_sb = wp.tile([P, NCH], FP16)
        nc.scalar.dma_start(v_sb, vt[:])
        for c in range(NCH):
            nc.scalar.dma_start(w1_sb[:, c], w1r[c])

        # V replicated along a 128-wide M dim so the scores matmul outputs
        # 128 identical rows (replicated scores; Z then comes out per-partition).
        vrep = wp.tile([P, NCH, P], FP16)
        nc.vector.tensor_copy(vrep, v_sb[:, :, None].to_broadcast([P, NCH, P]))

        # SWDGE flood gate: the steady-state SWDGE h-tile prefetches would
        # otherwise start at t=8us and crush the startup-critical rings to
        # ~22GB/s (the tile scheduler hoists independent DMAs, so a plain
        # fence instruction gets reordered around). Binding mechanism:
        # occupy every "hT" pool buffer with a dummy tile and write a couple
        # of elements into it from hT1a -- the write depends on the sync
        # ring's last-but-one startup load, and the real h-tile DMAs reuse
        # these buffers, so their WAW buffer dependency delays them until
        # the startup loads are nearly done.
        gate_tiles = []
        for _ in range(4):
            dummy = tp.tile([P, TQ, NCH, P], FP16, tag="hT")
            gate_tiles.append(dummy)
        # first two buffers (used by q3, b1q0) release once the last
        # startup-critical load lands; the write for the other two is
        # emitted later, after b0-q0's exp
        for dummy in gate_tiles[:2]:
            nc.vector.tensor_copy(dummy[0:1, 0, 0, 0:2], hT1b[0:1, 0, 0, 0:2])

        # ---- main loop over batches ----
        first = True
        for _rep in range(reps):
          for b in range(NB):
              exp_rep = xp.tile([P, L], FP16, tag="exp")     # exp(scores), replicated on all partitions
              zsl = fin.tile([P, NQ], FP32, tag="zsl")       # per-quarter sum of exp(scores)
              ctx_sl = fin.tile([P, NCH, NQ], FP32, tag="ctxsl")  # per-quarter ctx partials
              enc_sb = ep.tile([P, NCH, L], FP16, tag="enc")  # enc_projT (with b1), fp16

              for q in range(NQ):
                  # h_encT quarter tile [dpart, t, dchunk, l128].
                  # First batch: quarters arrive as 2 half-tiles so the PE can
                  # start on the first half while the second is in flight
                  # (q0 on the sync HW ring; the rest via SWDGE, whose cold
                  # latency would otherwise stall the PE after q0).
                  pieces = None
                  if first and q == 0:
                      pieces = [hT0a, hT0b]
                  elif first and q == 1:
                      pieces = [hT1a, hT1b]
                  else:
                      hT = tp.tile([P, TQ, NCH, P], FP16, tag="hT")
                      nc.gpsimd.dma_start(hT, ht[b, q])

                  ps_sc = psS.tile([P, LQ], FP32, tag="sc")
                  htan = hp.tile([P, NCH, LQ], FP16, tag="htan")
                  for c in range(NCH):
                      ps = psA.tile([P, LQ], FP32, tag="mm")
                      if pieces is None:
                          for d in range(NCH):
                              nc.tensor.matmul(
                                  ps,
                                  lhsT=w1_sb[:, c, d, :],
                                  rhs=hT[:, :, d, :],
                                  start=(d == 0),
                                  stop=(d == NCH - 1),
                              )
                      else:
                          for j, piece in enumerate(pieces):
                              for d in range(NCH):
                                  nc.tensor.matmul(
                                      ps[:, ts(j, LQ // 2)],
                                      lhsT=w1_sb[:, c, d, :],
                                      rhs=piece[:, :, d, :],
                                      start=(d == 0),
                                      stop=(d == NCH - 1),
                                  )
                      # tanh(enc_projT + dec_proj + b1 + b2) on ACT, fused bias
                      nc.scalar.activation(htan[:, c, :], ps, Act.Tanh, bias=bias_sb[:, c, b : b + 1])
                      # evacuate enc_projT + b1 to fp16 SBUF (ACT only: keeps the
                      # PSUM-drain path off DVE, whose ctx bursts would stall PE)
                      dst = enc_sb[:, c, ds(q * LQ, LQ)]
                      nc.scalar.activation(dst, ps, Act.Identity, bias=b1_sb[:, c : c + 1])
                  # scores for the whole quarter, batched back-to-back at the
                  # end: interleaving one scores matmul per chunk exposed a
                  # weight-(re)load bubble (+77ns) at each main<->scores
                  # boundary -- 8 boundaries/quarter; this has 2
                  for c in range(NCH):
                      nc.tensor.matmul(
                          ps_sc, lhsT=vrep[:, c, :], rhs=htan[:, c, :],
                          start=(c == 0), stop=(c == NCH - 1),
                      )
                  # exp(scores) + per-partition Z sum, fused on ACT
                  nc.scalar.activation(
                      exp_rep[:, ds(q * LQ, LQ)], ps_sc, Act.Exp,
                      accum_out=zsl[:, q : q + 1],
                  )
                  if first:
                      # release the last two gated h-tile buffers only now
                      for dummy in gate_tiles[2:]:
                          nc.vector.tensor_copy(
                              dummy[0:1, 0, 0, 0:2], exp_rep[0:1, 0:2]
                          )
                  first = False
                  # ctx_unnorm[c] partial for this quarter (overlaps next quarter's MMs)
                  last = b == NB - 1 and q == NQ - 1 and _rep == reps - 1
                  with nc.allow_low_precision("fp16 product scratch; |ctx_unnorm|<~1e3"):
                      if last:
                          # final quarter: the fused STT runs at reduce-path
                          # speed (0.61us/chunk) and nothing overlaps it, so
                          # split the chain: 4 chunks as plain DVE products
                          # (0.35us) reduced on the otherwise-idle ACT engine
                          # via Identity+accum_out, 4 as fused STTs on DVE
                          for c in range(4, NCH):
                              scratch = sp.tile([P, LQ], FP16, tag="ttr")
                              nc.vector.tensor_tensor(
                                  scratch, enc_sb[:, c, ds(q * LQ, LQ)],
                                  exp_rep[:, ds(q * LQ, LQ)], Alu.mult,
                              )
                              junk = sp.tile([P, LQ], FP16, tag="junk")
                              nc.scalar.activation(
                                  junk, scratch, Act.Identity,
                                  accum_out=ctx_sl[:, c, q : q + 1],
                              )
                      for c in range(4 if last else 0, 4) if last else range(NCH):
                          pass
                      for c in range(4) if last else range(NCH):
                          scratch = sp.tile([P, LQ], FP16, tag="ttr")
                          nc.vector.scalar_tensor_tensor(
                              out=scratch,
                              in0=enc_sb[:, c, ds(q * LQ, LQ)],
                              scalar=1.0,
                              in1=exp_rep[:, ds(q * LQ, LQ)],
                              op0=Alu.mult,
                              op1=Alu.mult,
                              accum_out=ctx_sl[:, c, q : q + 1],
                          )

              # raw partials out; host sums quarters and divides by Z
              # (z first: its data is ready at exp time, before the ctx STTs)
              nc.sync.dma_start(zout[b], zsl)
              nc.sync.dma_start(out[b], ctx_sl)

    nc.finalize()
    return nc


def _prep_shared(W1, b1, W2, b2, V):
    f16 = np.float16
    # w1r[c][p_d, dc, e'] = W1T[dc*128+p_d, c*128+e']
    w1r = np.ascontiguousarray(
        W1.T.reshape(NCH, P, NCH, P).transpose(2, 1, 0, 3).astype(f16)
    )
    b1t = np.ascontiguousarray(b1.reshape(NCH, P).T.astype(np.float32))
    vt = np.ascontiguousarray(V.reshape(NCH, P).T.astype(f16))
    return w1r, b1t, vt


def _prep_ht(h_enc_slice):
    # ht[b, q, dp, t, dc, lp] = h_enc[b, q*LQ + t*P + lp, dc*P + dp] as fp16
    h16 = h_enc_slice.astype(np.float16)
    v = h16.reshape(NB, NQ, TQ, P, NCH, P).transpose(0, 1, 5, 2, 4, 3)
    return np.ascontiguousarray(v)


def kernel(h_enc, h_dec, W1, b1, W2, b2, V, bv):
    from concourse.bass_utils import run_bass_kernel_spmd

    h_enc = np.asarray(h_enc, dtype=np.float32)
    h_dec = np.asarray(h_dec, dtype=np.float32)
    W1 = np.asarray(W1, dtype=np.float32)
    b1 = np.asarray(b1, dtype=np.float32)
    W2 = np.asarray(W2, dtype=np.float32)
    b2 = np.asarray(b2, dtype=np.float32)
    V = np.asarray(V, dtype=np.float32)

    if "nc" not in _cache:
        _cache["nc"] = _build()
    nc = _cache["nc"]

    w1r, b1t, vt = _prep_shared(W1, b1, W2, b2, V)
    # dec_proj + b1 + b2 on host (tiny GEMM)
    bias_full = h_dec @ W2.T + b2 + b1  # [B, D] fp32

    in_maps = []
    for core in range(NCORES):
        sl = slice(core * NB, (core + 1) * NB)
        biast = np.ascontiguousarray(
            bias_full[sl].T.reshape(NCH, P, NB).transpose(1, 0, 2).astype(np.float32)
        )
        in_maps.append(
            {
                "ht": _prep_ht(h_enc[sl]),
                "w1r": w1r,
                "biast": biast,
                "b1t": b1t,
                "vt": vt,
            }
        )

    res = run_bass_kernel_spmd(nc, in_maps, core_ids=list(range(NCORES)))
    outs = []
    for core in range(NCORES):
        cq = res.results[core]["ctx_out"].astype(np.float64)  # [NB, P, NCH, NQ]
        z = res.results[core]["z_out"].astype(np.float64)     # [NB, P, NQ]
        ctx = cq.sum(-1) / z.sum(-1)[:, :, None]              # [NB, P, NCH]
        outs.append(ctx.transpose(0, 2, 1).reshape(NB, D))    # e = c*128 + p
    return np.concatenate(outs, axis=0).astype(np.float32)


# revision 42
# speedup vs baseline: 1.0092x; 1.0092x over previous
"""Bahdanau-attention kernel for Trainium2 (8 NeuronCores, data-parallel over batch).

Computation (per batch b):
    enc_proj = h_enc @ W1.T + b1          # (L, D)   -- the big matmul
    dec_proj = h_dec @ W2.T + b2          # (D,)  -- computed on host (tiny)
    h        = tanh(enc_proj + dec_proj)  # (L, D)
    scores   = h @ V (+ bv)               # (L,)  -- bv cancels in softmax, dropped
    attn     = softmax(scores)            # no-max softmax: |scores| <= ||V||_1 ~ 16, exp is safe
    ctx      = attn @ enc_proj            # (D,)

Device layout: everything transposed ("T-space", e/d on partitions):
  - h_enc cast to fp16 AND pre-transposed on HOST into the exact SBUF
    layout [b, q, dpart, t, dchunk, l]; the device just streams one
    contiguous 1MB DMA per l-quarter (no on-device transpose)
  - dec_proj (+b1+b2) computed on host (67 MFLOP numpy) -> "biast" input;
    no W2/h_dec on device at all
  - q0/q1 tiles load as 2 half-tiles each on the otherwise-idle sync HW
    ring (low latency); steady-state tiles on the gpsimd/SWDGE ring whose
    spread-out traffic does not disturb the PE clock (HW-DGE bulk does),
    gated behind the startup-critical loads via dummy-buffer WAW deps
  - L processed in quarters (512): load -> 8x8 matmuls (PE) ->
    tanh+evac (ACT) -> scores (PE) -> exp (ACT) -> ctx partial (DVE)
  - W1 relaid per-output-chunk and loaded as per-chunk DMAs so chunk c's
    semaphore fires as soon as its own bytes land
  - tanh fused with (host dec_proj+b1+b2) bias on ACT; exp fused with
    Z-sum on ACT
  - scores via PE with V replicated to 128 rows -> replicated scores for free
  - ctx via one fused DVE scalar_tensor_tensor (product + free-axis
    accumulate) per (chunk, quarter) against the evacuated enc_projT
  - softmax division and final quarter-sums happen on the host (raw
    per-quarter ctx/Z partials are the kernel outputs)
"""

import numpy as np

B, L, D = 32, 2048, 1024
NCORES = 8
NB = B // NCORES  # batches per core
P = 128
NCH = D // P      # 8 chunks of the d/e dimension
NQ = 4            # l-quarters per batch
LQ = L // NQ      # 512
TQ = LQ // P      # 4 l-subtiles per quarter

_cache = {}


def _build(reps=1):
    import concourse.bass as bass
    import concourse.tile as tile
    from concourse import bacc, mybir
    from concourse.bass import ts, ds
    from contextlib import ExitStack

    FP16 = mybir.dt.float16
    FP32 = mybir.dt.float32
    Alu = mybir.AluOpType
    Act = mybir.ActivationFunctionType
    X = mybir.AxisListType.X

    nc = bacc.Bacc("TRN2", name="bahdanau_attn")

    # host-pretransposed encoder states: ht[b, q, dp, t, dc, l]
    ht = nc.dram_tensor("ht", [NB, NQ, P, TQ, NCH, P], FP16, kind="ExternalInput")
    # per-output-chunk weight layout: w1r[c][dpart, dchunk, e'] = W1T[d, c*128+e']
    w1r = nc.dram_tensor("w1r", [NCH, P, NCH, P], FP16, kind="ExternalInput")
    biast = nc.dram_tensor("biast", [P, NCH, NB], FP32, kind="ExternalInput")  # dec_proj+b1+b2
    b1t = nc.dram_tensor("b1t", [P, NCH], FP32, kind="ExternalInput")          # b1 as [p, chunk]
    vt = nc.dram_tensor("vt", [P, NCH], FP16, kind="ExternalInput")            # V as [p, chunk]
    # raw per-quarter ctx partials and Z partials; host does sum + divide
    out = nc.dram_tensor("ctx_out", [NB, P, NCH, NQ], FP32, kind="ExternalOutput")
    zout = nc.dram_tensor("z_out", [NB, P, NQ], FP32, kind="ExternalOutput")

    with tile.TileContext(nc) as tc, ExitStack() as ctx:
        wp = ctx.enter_context(tc.tile_pool(name="weights", bufs=1))
        # small buffer counts on the h-tile pools are load-bearing: the SWDGE
        # ring's transfers gate on buffer release, which keeps the early HBM
        # fabric dedicated to the startup-critical q0/q1/weight loads
        tpp = ctx.enter_context(tc.tile_pool(name="hTpieces", bufs=2))
        q1p = ctx.enter_context(tc.tile_pool(name="hTq1", bufs=1))
        tp = ctx.enter_context(tc.tile_pool(name="hT", bufs=4))
        ep = ctx.enter_context(tc.tile_pool(name="encproj", bufs=2))
        hp = ctx.enter_context(tc.tile_pool(name="htan", bufs=4))
        xp = ctx.enter_context(tc.tile_pool(name="exps", bufs=2))
        sp = ctx.enter_context(tc.tile_pool(name="scratch", bufs=3))
        fin = ctx.enter_context(tc.tile_pool(name="final", bufs=2))
        psA = ctx.enter_context(tc.tile_pool(name="psA", bufs=5, space="PSUM"))
        psS = ctx.enter_context(tc.tile_pool(name="psS", bufs=2, space="PSUM"))

        w1_sb = wp.tile([P, NCH, NCH, P], FP16)  # [dpart, cchunk, dchunk, e']

        # first quarter's h_encT tile in 2 pieces on the idle sync HW ring
        # q0 as 2 half-tiles + q1 as one tile on the sync HW ring (serial
        # FIFO; SWDGE's ~20us cold-start latency can't feed the PE this
        # early). q0's halves let the PE start ~3us sooner; q1 arrives
        # before the PE reaches it either way, so full-width is cheaper.
        hT0a = tpp.tile([P, TQ // 2, NCH, P], FP16, tag="hTa")
        hT0b = tpp.tile([P, TQ // 2, NCH, P], FP16, tag="hTb")
        hT1a = tpp.tile([P, TQ // 2, NCH, P], FP16, tag="hTa")
        hT1b = tpp.tile([P, TQ // 2, NCH, P], FP16, tag="hTb")
        nc.sync.dma_start(hT0a, ht[0, 0, :, ds(0, TQ // 2)])
        nc.sync.dma_start(hT0b, ht[0, 0, :, ds(TQ // 2, TQ // 2)])
        nc.sync.dma_start(hT1a, ht[0, 1, :, ds(0, TQ // 2)])
        nc.sync.dma_start(hT1b, ht[0, 1, :, ds(TQ // 2, TQ // 2)])

        # prologue loads (scalar ring), critical-path first; w1 as per-chunk
        # DMAs so chunk c's semaphore fires as soon as ITS bytes land (a
        # single bulk DMA would gate c=3..7 on the full 2MB)
        bias_sb = wp.tile([P, NCH, NB], FP32)
        nc.scalar.dma_start(bias_sb, biast[:])
        b1_sb = wp.tile([P, NCH], FP32)
        nc.scalar.dma_start(b1_sb, b1t[:])
        v_sb = wp.tile([P, NCH], FP16)
        nc.scalar.dma_start(v_sb, vt[:])
        for c in range(NCH):
            nc.scalar.dma_start(w1_sb[:, c], w1r[c])

        # V replicated along a 128-wide M dim so the scores matmul outputs
        # 128 identical rows (replicated scores; Z then comes out per-partition).
        vrep = wp.tile([P, NCH, P], FP16)
        nc.vector.tensor_copy(vrep, v_sb[:, :, None].to_broadcast([P, NCH, P]))

        # SWDGE flood gate: the steady-state SWDGE h-tile prefetches would
        # otherwise start at t=8us and crush the startup-critical rings to
        # ~22GB/s (the tile scheduler hoists independent DMAs, so a plain
        # fence instruction gets reordered around). Binding mechanism:
        # occupy every "hT" pool buffer with a dummy tile and write a couple
        # of elements into it from hT1a -- the write depends on the sync
        # ring's last-but-one startup load, and the real h-tile DMAs reuse
        # these buffers, so their WAW buffer dependency delays them until
        # the startup loads are nearly done.
        gate_tiles = []
        for _ in range(4):
            dummy = tp.tile([P, TQ, NCH, P], FP16, tag="hT")
            gate_tiles.append(dummy)
        # first two buffers (used by q3, b1q0) release once the last
        # startup-critical load lands; the write for the other two is
        # emitted later, after b0-q0's exp
        for dummy in gate_tiles[:2]:
            nc.vector.tensor_copy(dummy[0:1, 0, 0, 0:2], hT1b[0:1, 0, 0, 0:2])

        # ---- main loop over batches ----
        first = True
        for _rep in range(reps):
          for b in range(NB):
              exp_rep = xp.tile([P, L], FP16, tag="exp")     # exp(scores), replicated on all partitions
              zsl = fin.tile([P, NQ], FP32, tag="zsl")       # per-quarter sum of exp(scores)
              ctx_sl = fin.tile([P, NCH, NQ], FP32, tag="ctxsl")  # per-quarter ctx partials
              enc_sb = ep.tile([P, NCH, L], FP16, tag="enc")  # enc_projT (with b1), fp16

              for q in range(NQ):
                  # h_encT quarter tile [dpart, t, dchunk, l128].
                  # First batch: quarters arrive as 2 half-tiles so the PE can
                  # start on the first half while the second is in flight
                  # (q0 on the sync HW ring; the rest via SWDGE, whose cold
                  # latency would otherwise stall the PE after q0).
                  pieces = None
                  if first and q == 0:
                      pieces = [hT0a, hT0b]
                  elif first and q == 1:
                      pieces = [hT1a, hT1b]
                  else:
                      hT = tp.tile([P, TQ, NCH, P], FP16, tag="hT")
                      nc.gpsimd.dma_start(hT, ht[b, q])

                  ps_sc = psS.tile([P, LQ], FP32, tag="sc")
                  htan = hp.tile([P, NCH, LQ], FP16, tag="htan")
                  for c in range(NCH):
                      ps = psA.tile([P, LQ], FP32, tag="mm")
                      if pieces is None:
                          for d in range(NCH):
                              nc.tensor.matmul(
                                  ps,
                                  lhsT=w1_sb[:, c, d, :],
                                  rhs=hT[:, :, d, :],
                                  start=(d == 0),
                                  stop=(d == NCH - 1),
                              )
                      else:
                          for j, piece in enumerate(pieces):
                              for d in range(NCH):
                                  nc.tensor.matmul(
                                      ps[:, ts(j, LQ // 2)],
                                      lhsT=w1_sb[:, c, d, :],
                                      rhs=piece[:, :, d, :],
                                      start=(d == 0),
                                      stop=(d == NCH - 1),
                                  )
                      # tanh(enc_projT + dec_proj + b1 + b2) on ACT, fused bias
                      nc.scalar.activation(htan[:, c, :], ps, Act.Tanh, bias=bias_sb[:, c, b : b + 1])
                      # evacuate enc_projT + b1 to fp16 SBUF (ACT only: keeps the
                      # PSUM-drain path off DVE, whose ctx bursts would stall PE)
                      dst = enc_sb[:, c, ds(q * LQ, LQ)]
                      nc.scalar.activation(dst, ps, Act.Identity, bias=b1_sb[:, c : c + 1])
                  # scores for the whole quarter, batched back-to-back at the
                  # end: interleaving one scores matmul per chunk exposed a
                  # weight-(re)load bubble (+77ns) at each main<->scores
                  # boundary -- 8 boundaries/quarter; this has 2
                  for c in range(NCH):
                      nc.tensor.matmul(
                          ps_sc, lhsT=vrep[:, c, :], rhs=htan[:, c, :],
                          start=(c == 0), stop=(c == NCH - 1),
                      )
                  # exp(scores) + per-partition Z sum, fused on ACT
                  nc.scalar.activation(
                      exp_rep[:, ds(q * LQ, LQ)], ps_sc, Act.Exp,
                      accum_out=zsl[:, q : q + 1],
                  )
                  if first:
                      # release the last two gated h-tile buffers only now
                      for dummy in gate_tiles[2:]:
                          nc.vector.tensor_copy(
                              dummy[0:1, 0, 0, 0:2], exp_rep[0:1, 0:2]
                          )
                  first = False
                  # ctx_unnorm[c] partial for this quarter (overlaps next quarter's MMs)
                  with nc.allow_low_precision("fp16 product scratch; |ctx_unnorm|<~1e3"):
                      for c in range(NCH):
                          scratch = sp.tile([P, LQ], FP16, tag="ttr")
                          nc.vector.scalar_tensor_tensor(
                              out=scratch,
                              in0=enc_sb[:, c, ds(q * LQ, LQ)],
                              scalar=1.0,
                              in1=exp_rep[:, ds(q * LQ, LQ)],
                              op0=Alu.mult,
                              op1=Alu.mult,
                              accum_out=ctx_sl[:, c, q : q + 1],
                          )

              # raw partials out; host sums quarters and divides by Z
              # (z first: its data is ready at exp time, before the ctx STTs)
              nc.sync.dma_start(zout[b], zsl)
              nc.sync.dma_start(out[b], ctx_sl)

    nc.finalize()
    return nc


def _prep_shared(W1, b1, W2, b2, V):
    f16 = np.float16
    # w1r[c][p_d, dc, e'] = W1T[dc*128+p_d, c*128+e']
    w1r = np.ascontiguousarray(
        W1.T.reshape(NCH, P, NCH, P).transpose(2, 1, 0, 3).astype(f16)
    )
    b1t = np.ascontiguousarray(b1.reshape(NCH, P).T.astype(np.float32))
    vt = np.ascontiguousarray(V.reshape(NCH, P).T.astype(f16))
    return w1r, b1t, vt


def _prep_ht(h_enc_slice):
    # ht[b, q, dp, t, dc, lp] = h_enc[b, q*LQ + t*P + lp, dc*P + dp] as fp16
    h16 = h_enc_slice.astype(np.float16)
    v = h16.reshape(NB, NQ, TQ, P, NCH, P).transpose(0, 1, 5, 2, 4, 3)
    return np.ascontiguousarray(v)


def kernel(h_enc, h_dec, W1, b1, W2, b2, V, bv):
    from concourse.bass_utils import run_bass_kernel_spmd

    h_enc = np.asarray(h_enc, dtype=np.float32)
    h_dec = np.asarray(h_dec, dtype=np.float32)
    W1 = np.asarray(W1, dtype=np.float32)
    b1 = np.asarray(b1, dtype=np.float32)
    W2 = np.asarray(W2, dtype=np.float32)
    b2 = np.asarray(b2, dtype=np.float32)
    V = np.asarray(V, dtype=np.float32)

    if "nc" not in _cache:
        _cache["nc"] = _build()
    nc = _cache["nc"]

    w1r, b1t, vt = _prep_shared(W1, b1, W2, b2, V)
    # dec_proj + b1 + b2 on host (tiny GEMM)
    bias_full = h_dec @ W2.T + b2 + b1  # [B, D] fp32

    in_maps = []
    for core in range(NCORES):
        sl = slice(core * NB, (core + 1) * NB)
        biast = np.ascontiguousarray(
            bias_full[sl].T.reshape(NCH, P, NB).transpose(1, 0, 2).astype(np.float32)
        )
        in_maps.append(
            {
                "ht": _prep_ht(h_enc[sl]),
                "w1r": w1r,
                "biast": biast,
                "b1t": b1t,
                "vt": vt,
            }
        )

    res = run_bass_kernel_spmd(nc, in_maps, core_ids=list(range(NCORES)))
    outs = []
    for core in range(NCORES):
        cq = res.results[core]["ctx_out"].astype(np.float64)  # [NB, P, NCH, NQ]
        z = res.results[core]["z_out"].astype(np.float64)     # [NB, P, NQ]
        ctx = cq.sum(-1) / z.sum(-1)[:, :, None]              # [NB, P, NCH]
        outs.append(ctx.transpose(0, 2, 1).reshape(NB, D))    # e = c*128 + p
    return np.concatenate(outs, axis=0).astype(np.float32)
